# revision 15
# baseline (speedup 1.0000x reference)
"""MeshFC kernel for 8x TRN2 NeuronCores.

Computes: out = inputs @ w + biases, where
  w[i,o] = ||in_pos[i]-out_pos[o]|| - ||init_in_pos[i]-init_out_pos[o]||

Sharding: tensor-parallel on the output dim (8 x 1024 columns). Each core
generates its weight column block on-chip, then runs the main
[4096,2048]x[2048,1024] matmul in fp16 (1 cycle/row, 512-col PSUM tiles).

The weight splits as w = dC - dI where dC depends on the perturbed
positions and dI only on the init positions. dI = sqrt(dI0^2 + eps) is
computed host-side in float64 and shipped as fp32 (16 bits would not
do: dI quantization error aggregates over K=2048 into ~3e-2 rel
error). dC^2 is generated on-device with the augmented-inner-product
identity dist^2 = ||a||^2 - 2 a.b + ||b||^2: each fp32 augmented
coordinate is split into two fp16 parts (11+11 mantissa bits) and the
cross-products (hh, hm, mh, mm) become a single K=29 fp16 matmul at
1 cycle/row. fp16 products are exact and accumulate in fp32 PSUM, so
dC^2 comes out accurate to ~2e-5 absolute, which matters for
near-coincident point pairs where sqrt amplifies absolute error.
(fp32r cannot be used here: its datapath rounds the large intermediate
products to fp22, giving ~8e-3 error on dist^2.) An eps coordinate
keeps PSUM positive (no clamp before sqrt); the same eps inside the
host dI cancels the resulting bias to first order.

Schedule: every main matmul for an output half needs that half of w
for ALL 16 k-tiles, so the weight-gen -> sqrt -> subtract chain gates
the main phase. To shorten the gate, only the oh0 half is generated
up front (per k-tile: one 512-col matmul, one sqrt, one sub — the subs
alternating DVE/GPSIMD so neither serial chain paces the gate). The
entire oh1 weight-gen is emitted in small chunks between the first
batch tiles of the oh0 main pass: its matmuls add ~7us to the 112us
pass and its sqrt/subs run on the otherwise-idle ScalarE/DVE under
it. x tiles are streamed twice (once per half) rather than keeping
all 16 MB resident.

Bias is added host-side (free O(N) pass on the gathered output).
Host side pre-transposes/pre-tiles inputs so every DMA is contiguous,
and concatenates the 8 per-core [4096,1024] outputs.
"""

import os
from contextlib import ExitStack

import numpy as np

NUM_IN, NUM_OUT, SD, BATCH = 2048, 8192, 5, 4096
N_CORES = 8
O_SHARD = NUM_OUT // N_CORES  # 1024
B_TILES = BATCH // 128  # 32
K_TILES = NUM_IN // 128  # 16
O_HALVES = O_SHARD // 512  # 2
KAUG = 29  # 7 aug coords x 4 fp16 cross-products + eps coordinate
EPS = 1e-4
# oh0 subs routed to GPSIMD for these k-tiles (GPSIMD is ~1.7x slower
# per op than DVE, so it gets the smaller share of the gate-critical set)
GP_SET = frozenset({2, 5, 8, 11, 14})

_CACHE = {}


def _build_bass(variant=""):
    import concourse.mybir as mybir
    from concourse import bacc
    from concourse.tile import TileContext

    fp32 = mybir.dt.float32
    fp16 = mybir.dt.float16

    nc = bacc.Bacc("TRN2", name="meshfc")

    xT = nc.dram_tensor("xT", [B_TILES, 128, NUM_IN], fp16, kind="ExternalInput")
    # packed [UC | VC] along the free axis
    AB_W = NUM_IN + O_SHARD
    ab = nc.dram_tensor("ab", [KAUG, AB_W], fp16, kind="ExternalInput")
    # host dI, laid out [oh][kt] so each (oh, kt) chunk is one contiguous DMA
    dI = nc.dram_tensor("dI", [128, O_HALVES * K_TILES * 512], fp32,
                        kind="ExternalInput")
    out = nc.dram_tensor("out", [BATCH, O_SHARD], fp32, kind="ExternalOutput")

    with ExitStack() as ctx:
        tc = ctx.enter_context(TileContext(nc))
        const = ctx.enter_context(tc.tile_pool(name="const", bufs=1))
        pps = ctx.enter_context(tc.tile_pool(name="pps", bufs=2, space="PSUM"))
        s0p = ctx.enter_context(tc.tile_pool(name="s0p", bufs=4))
        s1p = ctx.enter_context(tc.tile_pool(name="s1p", bufs=4))
        d0p = ctx.enter_context(tc.tile_pool(name="d0p", bufs=1))
        d1p = ctx.enter_context(tc.tile_pool(name="d1p", bufs=1))
        xpool = ctx.enter_context(tc.tile_pool(name="xp", bufs=3))
        opool = ctx.enter_context(tc.tile_pool(name="op", bufs=4))

        # --- constants ---
        ab_sb = const.tile([KAUG, AB_W], fp16, name="ab_sb")
        # chunk by partition ranges: each partition row is one DMA
        # descriptor and a single queue is descriptor-rate/bandwidth
        # bound, so thin chunks fan out across queues.
        for p0 in range(0, KAUG, 8):
            p1 = min(p0 + 8, KAUG)
            nc.sync.dma_start(out=ab_sb[p0:p1, :], in_=ab[p0:p1, :])
        uC_sb = ab_sb[:, 0:NUM_IN]
        vC_sb = ab_sb[:, NUM_IN:AB_W]

        # resident weight block as 32 per-(oh,kt) tiles (4 MB total):
        # separate tiles give the scheduler exact write->read deps, so a
        # main matmul only waits for the one w slice it reads (a single
        # fused tile made every main matmul wait for ALL 32 sub writes).
        w_t = {}
        for oh in range(O_HALVES):
            for kt in range(K_TILES):
                w_t[oh, kt] = const.tile([128, 512], fp16, name=f"w{oh}_{kt}")

        # stream host dI: 4 k-tiles per dma_start (issue costs ~0.6us
        # on the Sync queue, so fewer+bigger issues; [128, x] transfers
        # fan out across all 16 DMA queues on their own)
        dI_t = {}
        for oh in range(O_HALVES):
            pool = d0p if oh == 0 else d1p
            for kt0 in range(0, K_TILES, 4):
                dt_ = pool.tile([128, 4, 512], fp32, name=f"dI{oh}_{kt0}")
                base = (oh * K_TILES + kt0) * 512
                nc.sync.dma_start(out=dt_, in_=dI[:, base : base + 4 * 512])
                for j in range(4):
                    dI_t[oh, kt0 + j] = dt_[:, j, :]

        def wgen(kt, oh, eng):
            ksl = slice(kt * 128, (kt + 1) * 128)
            osl = slice(oh * 512, (oh + 1) * 512)
            tag = f"pw{oh}"
            ps = pps.tile([128, 512], fp32, tag=tag, bufs=2, name=tag)
            nc.tensor.matmul(ps, uC_sb[:, ksl], vC_sb[:, osl],
                             start=True, stop=True)
            # PSUM is >= eps - O(2e-5) > 0 by construction: sqrt straight
            # out of PSUM, then w = dC - dI on DVE/GPSIMD.
            sp = s0p if oh == 0 else s1p
            s = sp.tile([128, 512], fp32, name=f"s{oh}")
            nc.scalar.sqrt(s, ps)
            eng.tensor_sub(w_t[oh, kt], s, dI_t[oh, kt])

        # --- oh0 weight-gen: the gate for the first main pass ---
        for kt in range(K_TILES):
            wgen(kt, 0, nc.gpsimd if kt in GP_SET else nc.vector)

        # --- main passes: all batch tiles for oh0, then for oh1.
        # oh1 weight-gen rides inside the first batch tiles of the oh0
        # pass: +213ns per PE matmul there, sqrt/subs under the pass.
        for oh in range(O_HALVES):
            osl = slice(oh * 512, (oh + 1) * 512)
            for bt in range(B_TILES):
                if oh == 0 and bt < 8:
                    for kt in range(bt * 2, bt * 2 + 2):
                        wgen(kt, 1, nc.vector)
                xt = xpool.tile([128, NUM_IN], fp16, name="xt")
                nc.sync.dma_start(out=xt, in_=xT[bt])
                ot = opool.tile([128, 512], fp32, name="ot")
                # pre-touch: absorbs the out-DMA slot-release wait on
                # ScalarE so the drain copies stay within HW wait slots
                nc.scalar.mul(ot[0:1, 0:1], ot[0:1, 0:1], 0.0)
                ps = pps.tile([128, 512], fp32, tag="pm", bufs=4, name="ps")
                for kt in range(K_TILES):
                    nc.tensor.matmul(
                        ps,
                        xt[:, kt * 128 : (kt + 1) * 128],
                        w_t[oh, kt],
                        start=(kt == 0),
                        stop=(kt == K_TILES - 1),
                    )
                nc.scalar.copy(ot, ps)
                nc.sync.dma_start(
                    out=out[bt * 128 : (bt + 1) * 128, osl], in_=ot
                )

    nc.finalize()
    return nc


def _split2(a32):
    """Split fp32 -> (hi, mid) fp16 parts; hi+mid covers 22 mantissa bits."""
    h = a32.astype(np.float16).astype(np.float32)
    m = (a32 - h).astype(np.float16).astype(np.float32)
    return h, m


def _aug_a(p64):  # in-side points [N,5] -> [N,7] fp32 aug
    return np.concatenate(
        [p64, (p64 * p64).sum(1)[:, None], np.ones((len(p64), 1))], 1
    ).astype(np.float32)


def _aug_b(q64):  # out-side points [N,5] -> [N,7] fp32 aug
    return np.concatenate(
        [-2.0 * q64, np.ones((len(q64), 1)), (q64 * q64).sum(1)[:, None]], 1
    ).astype(np.float32)


def _split_u(A):  # [N,7] -> [N,29]: [h,h,m,m, sqrt(eps)] (pairs w/ _split_v)
    h, m = _split2(A)
    e = np.full((len(A), 1), np.sqrt(EPS), np.float32)
    return np.concatenate([h, h, m, m, e], 1)


def _split_v(B):  # [N,7] -> [N,29]: [h,m,h,m, sqrt(eps)]
    h, m = _split2(B)
    e = np.full((len(B), 1), np.sqrt(EPS), np.float32)
    return np.concatenate([h, m, h, m, e], 1)


def _init_dists(a0, b0):  # float64 [2048,5],[8192,5] -> fp32 [2048,8192]
    d2 = ((a0 * a0).sum(1)[:, None] - 2.0 * (a0 @ b0.T)
          + (b0 * b0).sum(1)[None, :])
    return np.sqrt(np.maximum(d2, 0.0) + EPS).astype(np.float32)


def _prep_inputs(inputs, init_in_pos, init_out_pos, in_pos, out_pos, biases):
    x = np.ascontiguousarray(np.asarray(inputs, dtype=np.float32))
    a = np.asarray(in_pos, dtype=np.float64).reshape(NUM_IN, SD)
    a0 = np.asarray(init_in_pos, dtype=np.float64).reshape(NUM_IN, SD)
    b = np.asarray(out_pos, dtype=np.float64).reshape(NUM_OUT, SD)
    b0 = np.asarray(init_out_pos, dtype=np.float64).reshape(NUM_OUT, SD)
    bias = np.asarray(biases, dtype=np.float32).reshape(NUM_OUT)

    # [bt, p, kt*128+b'] = x[bt*128+b', kt*128+p]
    xT = np.ascontiguousarray(
        x.reshape(B_TILES, 128, K_TILES, 128)
        .transpose(0, 3, 2, 1)
        .astype(np.float16)
    ).reshape(B_TILES, 128, NUM_IN)

    uC = _split_u(_aug_a(a)).T  # [29, 2048]
    vC_full = _split_v(_aug_b(b)).T  # [29, 8192]
    dI_full = _init_dists(a0, b0)  # [2048, 8192] fp32

    in_maps = []
    for c in range(N_CORES):
        sl = slice(c * O_SHARD, (c + 1) * O_SHARD)
        ab = np.ascontiguousarray(
            np.concatenate([uC, vC_full[:, sl]], axis=1)
        ).astype(np.float16)
        # dI[p, (oh*K_TILES + kt)*512 + o] = dI_full[kt*128+p, c*1024 + oh*512 + o]
        dIc = np.ascontiguousarray(
            dI_full[:, sl]                         # [2048, 1024]
            .reshape(K_TILES, 128, O_HALVES, 512)  # [kt, p, oh, o]
            .transpose(1, 2, 0, 3)                 # [p, oh, kt, o]
            .reshape(128, O_HALVES * K_TILES * 512)
        )
        in_maps.append({"xT": xT, "ab": ab, "dI": dIc})
    return in_maps, bias


def _run(in_maps, trace=False):
    from concourse.bass_utils import run_bass_kernel_spmd

    if "nc" not in _CACHE:
        _CACHE["nc"] = _build_bass()
    nc = _CACHE["nc"]
    res = run_bass_kernel_spmd(
        nc, in_maps, core_ids=list(range(N_CORES)), trace=trace
    )
    outs = [r["out"] for r in res.results]
    return np.concatenate(outs, axis=1), res


def kernel(**inputs) -> np.ndarray:
    in_maps, bias = _prep_inputs(**inputs)
    out, _ = _run(in_maps, trace=bool(os.environ.get("MESHFC_TRACE")))
    return out + bias[None, :]


# revision 16
# speedup vs baseline: 1.1653x; 1.1653x over previous
"""MeshFC kernel for 8x TRN2 NeuronCores.

Computes: out = inputs @ w + biases, where
  w[i,o] = ||in_pos[i]-out_pos[o]|| - ||init_in_pos[i]-init_out_pos[o]||

Sharding: tensor-parallel on the output dim (8 x 1024 columns). Each core
generates its weight column block on-chip, then runs the main
[4096,2048]x[2048,1024] matmul in fp16 (1 cycle/row, 512-col PSUM tiles).

The weight splits as w = dC - dI where dC depends on the perturbed
positions and dI only on the init positions. dI = sqrt(dI0^2 + eps) is
computed host-side in float64 and shipped as fp32 (16 bits would not
do: dI quantization error aggregates over K=2048 into ~3e-2 rel
error). dC^2 is generated on-device with the augmented-inner-product
identity dist^2 = ||a||^2 - 2 a.b + ||b||^2: each fp32 augmented
coordinate is split into two fp16 parts (11+11 mantissa bits) and the
cross-products (hh, hm, mh, mm) become a single K=29 fp16 matmul at
1 cycle/row. fp16 products are exact and accumulate in fp32 PSUM, so
dC^2 comes out accurate to ~2e-5 absolute, which matters for
near-coincident point pairs where sqrt amplifies absolute error.
(fp32r cannot be used here: its datapath rounds the large intermediate
products to fp22, giving ~8e-3 error on dist^2.) An eps coordinate
keeps PSUM positive (no clamp before sqrt); the same eps inside the
host dI cancels the resulting bias to first order.

Schedule: every main matmul for an output half needs that half of w
for ALL 16 k-tiles, so the weight-gen -> sqrt -> subtract chain gates
the main phase. To shorten the gate, only the oh0 half is generated
up front (per k-tile: one 512-col matmul, one sqrt, one sub — the subs
alternating DVE/GPSIMD so neither serial chain paces the gate). The
entire oh1 weight-gen is emitted in small chunks between the first
batch tiles of the oh0 main pass: its matmuls add ~7us to the 112us
pass and its sqrt/subs run on the otherwise-idle ScalarE/DVE under
it. x tiles are streamed twice (once per half) rather than keeping
all 16 MB resident.

Bias is added host-side (free O(N) pass on the gathered output).
Host side pre-transposes/pre-tiles inputs so every DMA is contiguous,
and concatenates the 8 per-core [4096,1024] outputs.
"""

import os
from contextlib import ExitStack

import numpy as np

NUM_IN, NUM_OUT, SD, BATCH = 2048, 8192, 5, 4096
N_CORES = 8
O_SHARD = NUM_OUT // N_CORES  # 1024
B_TILES = BATCH // 128  # 32
K_TILES = NUM_IN // 128  # 16
O_HALVES = O_SHARD // 512  # 2
KAUG = 29  # 7 aug coords x 4 fp16 cross-products + eps coordinate
EPS = 1e-4
# oh0 subs routed to GPSIMD for these k-tiles (GPSIMD is ~1.7x slower
# per op than DVE, so it gets the smaller share of the gate-critical set)
GP_SET = frozenset()

_CACHE = {}


def _build_bass(variant=""):
    import concourse.mybir as mybir
    from concourse import bacc
    from concourse.tile import TileContext

    fp32 = mybir.dt.float32
    fp16 = mybir.dt.float16

    nc = bacc.Bacc("TRN2", name="meshfc")

    xT = nc.dram_tensor("xT", [B_TILES, 128, NUM_IN], fp16, kind="ExternalInput")
    # packed [UC | VC] along the free axis
    AB_W = NUM_IN + O_SHARD
    ab = nc.dram_tensor("ab", [KAUG, AB_W], fp16, kind="ExternalInput")
    # host dI, laid out [oh][kt] so each (oh, kt) chunk is one contiguous DMA
    dI = nc.dram_tensor("dI", [128, O_HALVES * K_TILES * 512], fp32,
                        kind="ExternalInput")
    out = nc.dram_tensor("out", [BATCH, O_SHARD], fp32, kind="ExternalOutput")

    with ExitStack() as ctx:
        tc = ctx.enter_context(TileContext(nc))
        const = ctx.enter_context(tc.tile_pool(name="const", bufs=1))
        pps = ctx.enter_context(tc.tile_pool(name="pps", bufs=2, space="PSUM"))
        s0p = ctx.enter_context(tc.tile_pool(name="s0p", bufs=4))
        s1p = ctx.enter_context(tc.tile_pool(name="s1p", bufs=4))
        d0p = ctx.enter_context(tc.tile_pool(name="d0p", bufs=1))
        d1p = ctx.enter_context(tc.tile_pool(name="d1p", bufs=1))
        xpool = ctx.enter_context(tc.tile_pool(name="xp", bufs=3))
        opool = ctx.enter_context(tc.tile_pool(name="op", bufs=4))

        # --- constants ---
        ab_sb = const.tile([KAUG, AB_W], fp16, name="ab_sb")
        # chunk by partition ranges: each partition row is one DMA
        # descriptor and a single queue is descriptor-rate/bandwidth
        # bound, so thin chunks fan out across queues.
        for p0 in range(0, KAUG, 8):
            p1 = min(p0 + 8, KAUG)
            nc.sync.dma_start(out=ab_sb[p0:p1, :], in_=ab[p0:p1, :])
        uC_sb = ab_sb[:, 0:NUM_IN]
        vC_sb = ab_sb[:, NUM_IN:AB_W]

        # resident weight block as 32 per-(oh,kt) tiles (4 MB total):
        # separate tiles give the scheduler exact write->read deps, so a
        # main matmul only waits for the one w slice it reads (a single
        # fused tile made every main matmul wait for ALL 32 sub writes).
        w_t = {}
        for oh in range(O_HALVES):
            for kt in range(K_TILES):
                w_t[oh, kt] = const.tile([128, 512], fp16, name=f"w{oh}_{kt}")

        # stream host dI: 4 k-tiles per dma_start (issue costs ~0.6us
        # on the Sync queue, so fewer+bigger issues; [128, x] transfers
        # fan out across all 16 DMA queues on their own)
        dI_t = {}
        for oh in range(O_HALVES):
            pool = d0p if oh == 0 else d1p
            for kt0 in range(0, K_TILES, 4):
                dt_ = pool.tile([128, 4, 512], fp32, name=f"dI{oh}_{kt0}")
                base = (oh * K_TILES + kt0) * 512
                nc.sync.dma_start(out=dt_, in_=dI[:, base : base + 4 * 512])
                for j in range(4):
                    dI_t[oh, kt0 + j] = dt_[:, j, :]

        def wgen(kt, oh, eng):
            ksl = slice(kt * 128, (kt + 1) * 128)
            osl = slice(oh * 512, (oh + 1) * 512)
            tag = f"pw{oh}"
            ps = pps.tile([128, 512], fp32, tag=tag, bufs=3 if oh == 0 else 2, name=tag)
            nc.tensor.matmul(ps, uC_sb[:, ksl], vC_sb[:, osl],
                             start=True, stop=True)
            # PSUM is >= eps - O(2e-5) > 0 by construction: sqrt straight
            # out of PSUM, then w = dC - dI on DVE/GPSIMD.
            sp = s0p if oh == 0 else s1p
            s = sp.tile([128, 512], fp32, name=f"s{oh}")
            nc.scalar.sqrt(s, ps)
            eng.tensor_sub(w_t[oh, kt], s, dI_t[oh, kt])

        # --- oh0 weight-gen: the gate for the first main pass ---
        for kt in range(K_TILES):
            wgen(kt, 0, nc.gpsimd if kt in GP_SET else nc.vector)

        # --- main passes: all batch tiles for oh0, then for oh1.
        # oh1 weight-gen rides inside the first batch tiles of the oh0
        # pass: +213ns per PE matmul there, sqrt/subs under the pass.
        for oh in range(O_HALVES):
            osl = slice(oh * 512, (oh + 1) * 512)
            for bt in range(B_TILES):
                if oh == 0 and bt < 8:
                    for kt in range(bt * 2, bt * 2 + 2):
                        wgen(kt, 1, nc.vector)
                xt = xpool.tile([128, NUM_IN], fp16, name="xt")
                nc.sync.dma_start(out=xt, in_=xT[bt])
                ot = opool.tile([128, 512], fp32, name="ot")
                # pre-touch: absorbs the out-DMA slot-release wait on
                # ScalarE so the drain copies stay within HW wait slots
                nc.scalar.mul(ot[0:1, 0:1], ot[0:1, 0:1], 0.0)
                ps = pps.tile([128, 512], fp32, tag="pm", bufs=3, name="ps")
                for kt in range(K_TILES):
                    nc.tensor.matmul(
                        ps,
                        xt[:, kt * 128 : (kt + 1) * 128],
                        w_t[oh, kt],
                        start=(kt == 0),
                        stop=(kt == K_TILES - 1),
                    )
                nc.scalar.copy(ot, ps)
                nc.sync.dma_start(
                    out=out[bt * 128 : (bt + 1) * 128, osl], in_=ot
                )

    nc.finalize()
    return nc


def _split2(a32):
    """Split fp32 -> (hi, mid) fp16 parts; hi+mid covers 22 mantissa bits."""
    h = a32.astype(np.float16).astype(np.float32)
    m = (a32 - h).astype(np.float16).astype(np.float32)
    return h, m


def _aug_a(p64):  # in-side points [N,5] -> [N,7] fp32 aug
    return np.concatenate(
        [p64, (p64 * p64).sum(1)[:, None], np.ones((len(p64), 1))], 1
    ).astype(np.float32)


def _aug_b(q64):  # out-side points [N,5] -> [N,7] fp32 aug
    return np.concatenate(
        [-2.0 * q64, np.ones((len(q64), 1)), (q64 * q64).sum(1)[:, None]], 1
    ).astype(np.float32)


def _split_u(A):  # [N,7] -> [N,29]: [h,h,m,m, sqrt(eps)] (pairs w/ _split_v)
    h, m = _split2(A)
    e = np.full((len(A), 1), np.sqrt(EPS), np.float32)
    return np.concatenate([h, h, m, m, e], 1)


def _split_v(B):  # [N,7] -> [N,29]: [h,m,h,m, sqrt(eps)]
    h, m = _split2(B)
    e = np.full((len(B), 1), np.sqrt(EPS), np.float32)
    return np.concatenate([h, m, h, m, e], 1)


def _init_dists(a0, b0):  # float64 [2048,5],[8192,5] -> fp32 [2048,8192]
    d2 = ((a0 * a0).sum(1)[:, None] - 2.0 * (a0 @ b0.T)
          + (b0 * b0).sum(1)[None, :])
    return np.sqrt(np.maximum(d2, 0.0) + EPS).astype(np.float32)


def _prep_inputs(inputs, init_in_pos, init_out_pos, in_pos, out_pos, biases):
    x = np.ascontiguousarray(np.asarray(inputs, dtype=np.float32))
    a = np.asarray(in_pos, dtype=np.float64).reshape(NUM_IN, SD)
    a0 = np.asarray(init_in_pos, dtype=np.float64).reshape(NUM_IN, SD)
    b = np.asarray(out_pos, dtype=np.float64).reshape(NUM_OUT, SD)
    b0 = np.asarray(init_out_pos, dtype=np.float64).reshape(NUM_OUT, SD)
    bias = np.asarray(biases, dtype=np.float32).reshape(NUM_OUT)

    # [bt, p, kt*128+b'] = x[bt*128+b', kt*128+p]
    xT = np.ascontiguousarray(
        x.reshape(B_TILES, 128, K_TILES, 128)
        .transpose(0, 3, 2, 1)
        .astype(np.float16)
    ).reshape(B_TILES, 128, NUM_IN)

    uC = _split_u(_aug_a(a)).T  # [29, 2048]
    vC_full = _split_v(_aug_b(b)).T  # [29, 8192]
    dI_full = _init_dists(a0, b0)  # [2048, 8192] fp32

    in_maps = []
    for c in range(N_CORES):
        sl = slice(c * O_SHARD, (c + 1) * O_SHARD)
        ab = np.ascontiguousarray(
            np.concatenate([uC, vC_full[:, sl]], axis=1)
        ).astype(np.float16)
        # dI[p, (oh*K_TILES + kt)*512 + o] = dI_full[kt*128+p, c*1024 + oh*512 + o]
        dIc = np.ascontiguousarray(
            dI_full[:, sl]                         # [2048, 1024]
            .reshape(K_TILES, 128, O_HALVES, 512)  # [kt, p, oh, o]
            .transpose(1, 2, 0, 3)                 # [p, oh, kt, o]
            .reshape(128, O_HALVES * K_TILES * 512)
        )
        in_maps.append({"xT": xT, "ab": ab, "dI": dIc})
    return in_maps, bias


def _run(in_maps, trace=False):
    from concourse.bass_utils import run_bass_kernel_spmd

    if "nc" not in _CACHE:
        _CACHE["nc"] = _build_bass()
    nc = _CACHE["nc"]
    res = run_bass_kernel_spmd(
        nc, in_maps, core_ids=list(range(N_CORES)), trace=trace
    )
    outs = [r["out"] for r in res.results]
    return np.concatenate(outs, axis=1), res


def kernel(**inputs) -> np.ndarray:
    in_maps, bias = _prep_inputs(**inputs)
    out, _ = _run(in_maps, trace=bool(os.environ.get("MESHFC_TRACE")))
    return out + bias[None, :]


# revision 18
# speedup vs baseline: 1.1870x; 1.0187x over previous
"""MeshFC kernel for 8x TRN2 NeuronCores.

Computes: out = inputs @ w + biases, where
  w[i,o] = ||in_pos[i]-out_pos[o]|| - ||init_in_pos[i]-init_out_pos[o]||

Sharding: tensor-parallel on the output dim (8 x 1024 columns). Each core
generates its weight column block on-chip, then runs the main
[4096,2048]x[2048,1024] matmul in fp16 (1 cycle/row, 512-col PSUM tiles).

The weight splits as w = dC - dI where dC depends on the perturbed
positions and dI only on the init positions. dI = sqrt(dI0^2 + eps) is
computed host-side in float64 and shipped as fp32 (16 bits would not
do: dI quantization error aggregates over K=2048 into ~3e-2 rel
error). dC^2 is generated on-device with the augmented-inner-product
identity dist^2 = ||a||^2 - 2 a.b + ||b||^2: each fp32 augmented
coordinate is split into two fp16 parts (11+11 mantissa bits) and the
cross-products (hh, hm, mh, mm) become a single K=29 fp16 matmul at
1 cycle/row. fp16 products are exact and accumulate in fp32 PSUM, so
dC^2 comes out accurate to ~2e-5 absolute, which matters for
near-coincident point pairs where sqrt amplifies absolute error.
(fp32r cannot be used here: its datapath rounds the large intermediate
products to fp22, giving ~8e-3 error on dist^2.) An eps coordinate
keeps PSUM positive (no clamp before sqrt); the same eps inside the
host dI cancels the resulting bias to first order.

Schedule: every main matmul for an output half needs that half of w
for ALL 16 k-tiles, so the weight-gen -> sqrt -> subtract chain gates
the main phase. To shorten the gate, only the oh0 half is generated
up front (per k-tile: one 512-col matmul, one sqrt, one DVE sub). The
entire oh1 weight-gen is emitted in small chunks between the first
batch tiles of the oh0 main pass: its matmuls add ~7us to the 112us
pass and its sqrt/subs run on the otherwise-idle ScalarE/DVE under
it. x tiles are streamed twice (once per half) rather than keeping
all 16 MB resident.

Bias is added host-side (free O(N) pass on the gathered output).
Host side pre-transposes/pre-tiles inputs so every DMA is contiguous,
and concatenates the 8 per-core [4096,1024] outputs.
"""

import os
from contextlib import ExitStack

import numpy as np

NUM_IN, NUM_OUT, SD, BATCH = 2048, 8192, 5, 4096
N_CORES = 8
O_SHARD = NUM_OUT // N_CORES  # 1024
B_TILES = BATCH // 128  # 32
K_TILES = NUM_IN // 128  # 16
O_HALVES = O_SHARD // 512  # 2
KAUG = 29  # 7 aug coords x 4 fp16 cross-products + eps coordinate
EPS = 1e-4
# k-tiles whose gate subs route to GPSIMD. Empty: GPSIMD's subs measured
# ~1.7x slower than DVE and occasionally stall ~40us (microcode reload),
# so all subs run on DVE.
GP_SET = frozenset()

_CACHE = {}


def _build_bass(variant=""):
    import concourse.mybir as mybir
    from concourse import bacc
    from concourse.tile import TileContext

    fp32 = mybir.dt.float32
    fp16 = mybir.dt.float16

    nc = bacc.Bacc("TRN2", name="meshfc")

    xT = nc.dram_tensor("xT", [B_TILES, 128, NUM_IN], fp16, kind="ExternalInput")
    # packed [UC | VC] along the free axis
    AB_W = NUM_IN + O_SHARD
    ab = nc.dram_tensor("ab", [KAUG, AB_W], fp16, kind="ExternalInput")
    # host dI, laid out [oh][kt] so each (oh, kt) chunk is one contiguous DMA
    dI = nc.dram_tensor("dI", [128, O_HALVES * K_TILES * 512], fp32,
                        kind="ExternalInput")
    out = nc.dram_tensor("out", [BATCH, O_SHARD], fp32, kind="ExternalOutput")

    with ExitStack() as ctx:
        tc = ctx.enter_context(TileContext(nc))
        const = ctx.enter_context(tc.tile_pool(name="const", bufs=1))
        pps = ctx.enter_context(tc.tile_pool(name="pps", bufs=2, space="PSUM"))
        s0p = ctx.enter_context(tc.tile_pool(name="s0p", bufs=4))
        s1p = ctx.enter_context(tc.tile_pool(name="s1p", bufs=4))
        d0p = ctx.enter_context(tc.tile_pool(name="d0p", bufs=1))
        d1p = ctx.enter_context(tc.tile_pool(name="d1p", bufs=1))
        xpool = ctx.enter_context(tc.tile_pool(name="xp", bufs=3))
        opool = ctx.enter_context(tc.tile_pool(name="op", bufs=4))

        # --- constants ---
        ab_sb = const.tile([KAUG, AB_W], fp16, name="ab_sb")
        # chunk by partition ranges: each partition row is one DMA
        # descriptor and a single queue is descriptor-rate/bandwidth
        # bound, so thin chunks fan out across queues.
        for p0 in range(0, KAUG, 8):
            p1 = min(p0 + 8, KAUG)
            nc.sync.dma_start(out=ab_sb[p0:p1, :], in_=ab[p0:p1, :])
        uC_sb = ab_sb[:, 0:NUM_IN]
        vC_sb = ab_sb[:, NUM_IN:AB_W]

        # resident weight block as 32 per-(oh,kt) tiles (4 MB total):
        # separate tiles give the scheduler exact write->read deps, so a
        # main matmul only waits for the one w slice it reads (a single
        # fused tile made every main matmul wait for ALL 32 sub writes).
        w_t = {}
        for oh in range(O_HALVES):
            for kt in range(K_TILES):
                w_t[oh, kt] = const.tile([128, 512], fp16, name=f"w{oh}_{kt}")

        # stream host dI: 4 k-tiles per dma_start (issue costs ~0.6us
        # on the Sync queue, so fewer+bigger issues; [128, x] transfers
        # fan out across all 16 DMA queues on their own)
        dI_t = {}
        for oh in range(O_HALVES):
            pool = d0p if oh == 0 else d1p
            for kt0 in range(0, K_TILES, 4):
                dt_ = pool.tile([128, 4, 512], fp32, name=f"dI{oh}_{kt0}")
                base = (oh * K_TILES + kt0) * 512
                nc.sync.dma_start(out=dt_, in_=dI[:, base : base + 4 * 512])
                for j in range(4):
                    dI_t[oh, kt0 + j] = dt_[:, j, :]

        def wgen(kt, oh, eng):
            ksl = slice(kt * 128, (kt + 1) * 128)
            osl = slice(oh * 512, (oh + 1) * 512)
            tag = f"pw{oh}"
            ps = pps.tile([128, 512], fp32, tag=tag, bufs=3 if oh == 0 else 2, name=tag)
            nc.tensor.matmul(ps, uC_sb[:, ksl], vC_sb[:, osl],
                             start=True, stop=True)
            # PSUM is >= eps - O(2e-5) > 0 by construction: sqrt straight
            # out of PSUM, then w = dC - dI on DVE/GPSIMD.
            sp = s0p if oh == 0 else s1p
            s = sp.tile([128, 512], fp32, name=f"s{oh}")
            nc.scalar.sqrt(s, ps)
            eng.tensor_sub(w_t[oh, kt], s, dI_t[oh, kt])

        # --- oh0 weight-gen: the gate for the first main pass ---
        for kt in range(K_TILES):
            wgen(kt, 0, nc.gpsimd if kt in GP_SET else nc.vector)

        # --- main passes: all batch tiles for oh0, then for oh1.
        # oh1 weight-gen rides inside the first batch tiles of the oh0
        # pass: +213ns per PE matmul there, sqrt/subs under the pass.
        for oh in range(O_HALVES):
            osl = slice(oh * 512, (oh + 1) * 512)
            for bt in range(B_TILES):
                if oh == 0 and bt < 8:
                    for kt in range(bt * 2, bt * 2 + 2):
                        wgen(kt, 1, nc.vector)
                xt = xpool.tile([128, NUM_IN], fp16, name="xt")
                nc.sync.dma_start(out=xt, in_=xT[bt])
                ot = opool.tile([128, 512], fp32, name="ot")
                # pre-touch: absorbs the out-DMA slot-release wait on
                # ScalarE so the drain copies stay within HW wait slots
                nc.scalar.mul(ot[0:1, 0:1], ot[0:1, 0:1], 0.0)
                ps = pps.tile([128, 512], fp32, tag="pm", bufs=3, name="ps")
                for kt in range(K_TILES):
                    nc.tensor.matmul(
                        ps,
                        xt[:, kt * 128 : (kt + 1) * 128],
                        w_t[oh, kt],
                        start=(kt == 0),
                        stop=(kt == K_TILES - 1),
                    )
                nc.scalar.copy(ot, ps)
                nc.sync.dma_start(
                    out=out[bt * 128 : (bt + 1) * 128, osl], in_=ot
                )

    nc.finalize()
    return nc


def _split2(a32):
    """Split fp32 -> (hi, mid) fp16 parts; hi+mid covers 22 mantissa bits."""
    h = a32.astype(np.float16).astype(np.float32)
    m = (a32 - h).astype(np.float16).astype(np.float32)
    return h, m


def _aug_a(p64):  # in-side points [N,5] -> [N,7] fp32 aug
    return np.concatenate(
        [p64, (p64 * p64).sum(1)[:, None], np.ones((len(p64), 1))], 1
    ).astype(np.float32)


def _aug_b(q64):  # out-side points [N,5] -> [N,7] fp32 aug
    return np.concatenate(
        [-2.0 * q64, np.ones((len(q64), 1)), (q64 * q64).sum(1)[:, None]], 1
    ).astype(np.float32)


def _split_u(A):  # [N,7] -> [N,29]: [h,h,m,m, sqrt(eps)] (pairs w/ _split_v)
    h, m = _split2(A)
    e = np.full((len(A), 1), np.sqrt(EPS), np.float32)
    return np.concatenate([h, h, m, m, e], 1)


def _split_v(B):  # [N,7] -> [N,29]: [h,m,h,m, sqrt(eps)]
    h, m = _split2(B)
    e = np.full((len(B), 1), np.sqrt(EPS), np.float32)
    return np.concatenate([h, m, h, m, e], 1)


def _init_dists(a0, b0):  # float64 [2048,5],[8192,5] -> fp32 [2048,8192]
    d2 = ((a0 * a0).sum(1)[:, None] - 2.0 * (a0 @ b0.T)
          + (b0 * b0).sum(1)[None, :])
    return np.sqrt(np.maximum(d2, 0.0) + EPS).astype(np.float32)


def _prep_inputs(inputs, init_in_pos, init_out_pos, in_pos, out_pos, biases):
    x = np.ascontiguousarray(np.asarray(inputs, dtype=np.float32))
    a = np.asarray(in_pos, dtype=np.float64).reshape(NUM_IN, SD)
    a0 = np.asarray(init_in_pos, dtype=np.float64).reshape(NUM_IN, SD)
    b = np.asarray(out_pos, dtype=np.float64).reshape(NUM_OUT, SD)
    b0 = np.asarray(init_out_pos, dtype=np.float64).reshape(NUM_OUT, SD)
    bias = np.asarray(biases, dtype=np.float32).reshape(NUM_OUT)

    # [bt, p, kt*128+b'] = x[bt*128+b', kt*128+p]
    xT = np.ascontiguousarray(
        x.reshape(B_TILES, 128, K_TILES, 128)
        .transpose(0, 3, 2, 1)
        .astype(np.float16)
    ).reshape(B_TILES, 128, NUM_IN)

    uC = _split_u(_aug_a(a)).T  # [29, 2048]
    vC_full = _split_v(_aug_b(b)).T  # [29, 8192]
    dI_full = _init_dists(a0, b0)  # [2048, 8192] fp32

    in_maps = []
    for c in range(N_CORES):
        sl = slice(c * O_SHARD, (c + 1) * O_SHARD)
        ab = np.ascontiguousarray(
            np.concatenate([uC, vC_full[:, sl]], axis=1)
        ).astype(np.float16)
        # dI[p, (oh*K_TILES + kt)*512 + o] = dI_full[kt*128+p, c*1024 + oh*512 + o]
        dIc = np.ascontiguousarray(
            dI_full[:, sl]                         # [2048, 1024]
            .reshape(K_TILES, 128, O_HALVES, 512)  # [kt, p, oh, o]
            .transpose(1, 2, 0, 3)                 # [p, oh, kt, o]
            .reshape(128, O_HALVES * K_TILES * 512)
        )
        in_maps.append({"xT": xT, "ab": ab, "dI": dIc})
    return in_maps, bias


def _run(in_maps, trace=False):
    from concourse.bass_utils import run_bass_kernel_spmd

    if "nc" not in _CACHE:
        _CACHE["nc"] = _build_bass()
    nc = _CACHE["nc"]
    res = run_bass_kernel_spmd(
        nc, in_maps, core_ids=list(range(N_CORES)), trace=trace
    )
    outs = [r["out"] for r in res.results]
    return np.concatenate(outs, axis=1), res


def kernel(**inputs) -> np.ndarray:
    in_maps, bias = _prep_inputs(**inputs)
    out, _ = _run(in_maps, trace=bool(os.environ.get("MESHFC_TRACE")))
    return out + bias[None, :]
